# revision 54
# baseline (speedup 1.0000x reference)
"""GrokkingSNN forward on 8 TRN2 NeuronCores — run-aligned, matmul-free
cur1 build, column-major chain with stores pipelined under it.

Math notes
----------
Reference loop (thr clamps absorbed; beta1=beta2=beta):
    m_t = beta*m_{t-1} + c - s_{t-1},  s_t = H(m_t - 1)
    out = W2 @ S + G*b2,  S = sum_t beta^(15-t) s_t
With q = kq*cur1 + (1 + kq*b1), kq = -1/((1-beta)*thr), and the premultiplied
state w = beta*z, qb = beta*q, the spike sum telescopes:
    S = beta^16*q - z16 - beta^16
so the device needs z16 (nonlinear part) while the beta^16*q term is LINEAR
in cur1 and is applied in output space.

x has only 97*97 distinct rows, so the net is evaluated on the (i, j) grid.
cur1[h, c] = A1T[i(c), h] + A2T[j(c), h] with i = c//97, j = c mod 97, hence
qb[:, c] = QA2[:, j] + QA1col(i): a 97-periodic tensor plus a per-run
per-partition column.  Each core takes 13 WHOLE runs of 97 columns (cores
overlap by one run so every core runs the identical run-aligned program), so
qb is built with NO matmuls/one-hots: run 0 via 4 DVE tensor_scalar adds
(so the chain starts ~1us after the first DMA receipt), runs 1-12 as 48
ScalarE activation-with-per-partition-bias ops that stay ~1us/run ahead of
the chain (GpSimd tensor_scalar measured ~2us/op — unusable).

The 15-step recurrence runs as six fused 8-ALU-stage custom-DVE ops
(OPEN, WGZ, ZGW, WGZ, ZGW, WGZ -> z16 fp16).  15 decisions x ~3 ALUs each
over 8 ALU stages/cycle makes ~5 cycles/element the hard DVE floor; the 2x
perf modes duplicate the per-element program across ALU stages, so 8-stage
programs can never engage them — the chain is the ~35us bottleneck and
everything else hides behind it.  Groups are COLUMN-major ([128, 4tiles, w]
strided APs over all hidden tiles) with graduated widths (r0-1 | r2-4 |
r5-8 | r9-12): each group completes whole psum chunks, whose W2 matmuls,
psum->f16 cast and HBM store then run DURING the chain.  Store descriptors
all funnel through one DMA engine (~45-100ns per partition row, regardless
of queue/layout — loads spread across 16 engines by destination partition,
stores do not), so hiding the first two chunk stores under the chain is the
only way to shorten the ~5us/chunk drain tail; only the last chunk pays it.

The linear beta^16*q part costs ONE extra fp16 matmul per psum chunk:
stationary [U2B.T ; U1B.T+consts] against a constant 0/1 matrix M
([j==row] rows and [run==row] rows), scheduled before any z16 exists.
Remaining fixed costs: ~4.7us first-DMA receipt latency and ~8us
runtime teardown (reset of all 256 semaphores + cross-core barriers).
"""

import os
import sys

import numpy as np

for _p in ("/opt/trn_rl_repo",):
    if _p not in sys.path and os.path.isdir(_p):
        sys.path.insert(0, _p)

P = 97          # vocab / output dim
H = 512         # hidden
NSTEPS = 15
NCORES = 8
NRUNS = 13      # 97-col runs per core
NLOC = NRUNS * P            # 1261 grid columns per core
HT = H // 128   # 4 hidden tiles
RUN_BASE = [(k * P) // NCORES for k in range(NCORES)]   # first run per core
CHUNKS = [(0, 5 * P), (5 * P, 4 * P), (9 * P, 4 * P)]  # psum col chunks

_CACHE = {}
_DVE_OPS = {}


def _register_dve_ops():
    """Register the three fused recurrence ops (documented extension
    path: define a DveOp and append to dve_ops.OPS)."""
    if _DVE_OPS:
        return _DVE_OPS
    import concourse.dve_ops as dvo
    from concourse.dve_spec import (Spec, Src0, Src1, C0, C1, C2,
                                    lower, _has_src1)
    from concourse.dve_uop import DveOpSpec

    def f32(a):
        return np.asarray(a, np.float32)

    # OPEN: in0 = qb (= beta*q).  s1 = [qb < c0]; z2 = qb*c1 - c2 - s1;
    # w2 = z2*c1; s2 = [w2 > qb]; z3 = w2 - s2; out = z3*c1   (c1 = beta)
    _s1 = Src0 < C0
    _z2 = (Src0 * C1 - C2) - _s1
    _w2 = _z2 * C1
    _z3 = _w2 - (_w2 > Src0)
    OPEN_spec = Spec(
        body=_z3 * C1,
        reference=lambda in0, in1, s0, s1, imm2: (lambda qb, s1v: (
            lambda z2: (lambda w2: (lambda z3: f32(z3 * s1))(
                f32(w2 - (w2 > qb))))(f32(z2 * s1)))(
            f32(f32(f32(qb * s1) - imm2) - s1v)))(
            f32(in0), f32(in0 < s0)),
    )

    # WGZ: in0 = w_t, in1 = qb: three spike decisions -> z_{t+3}
    _a2 = Src0 - (Src0 > Src1)
    _a3 = _a2 * C0
    _a5 = _a3 - (_a3 > Src1)
    _a6 = _a5 * C0
    WGZ_spec = Spec(
        body=_a6 - (_a6 > Src1),
        reference=lambda in0, in1, s0, s1, imm2: (lambda w, qb, b: (
            lambda z1: (lambda w1: (lambda z2: (lambda w2: f32(w2 - (w2 > qb)))(
                f32(z2 * b)))(f32(w1 - (w1 > qb))))(f32(z1 * b)))(
            f32(w - (w > qb))))(f32(in0), f32(in1), s0),
    )

    # ZGW: in0 = z_t, in1 = qb: two spike decisions -> w_{t+2}
    _b1 = Src0 * C0
    _b3 = _b1 - (_b1 > Src1)
    _b4 = _b3 * C0
    _b6 = _b4 - (_b4 > Src1)
    ZGW_spec = Spec(
        body=_b6 * C0,
        reference=lambda in0, in1, s0, s1, imm2: (lambda z, qb, b: (
            lambda w: (lambda z1: (lambda w1: (lambda z2: f32(z2 * b))(
                f32(w1 - (w1 > qb))))(f32(z1 * b)))(f32(w - (w > qb))))(
            f32(z * b)))(f32(in0), f32(in1), s0),
    )

    for name, spec in (("SNN_OPEN", OPEN_spec), ("SNN_WGZ", WGZ_spec),
                       ("SNN_ZGW", ZGW_spec)):
        if name not in dvo._SUB_OPCODE_FOR_NAME:
            row = dvo._CUSTOM_DVE_ROW_BASE + len(dvo.OPS)
            assert row < 0x20
            dvo._SUB_OPCODE_FOR_NAME[name] = row
            shas = {}
            for ver in ("v3", "v4"):
                uops = lower(spec, ver=ver)
                shas[ver] = DveOpSpec(name=name, opcode=row, uops=uops,
                                      rd1_en=_has_src1(spec)).sha(ver)
            op = dvo.DveOp(name, spec, subdim=False, uops_sha=shas)
            dvo.OPS.append(op)
            dvo.CUSTOM_DVE_SPECS[name] = spec
        _DVE_OPS[name] = next(o for o in dvo.OPS if o.name == name)
    return _DVE_OPS


def _build_bass(beta: float, thr: float):
    from concourse import bacc, mybir
    from concourse.tile import TileContext

    ops = _register_dve_ops()
    f32 = mybir.dt.float32
    f16 = mybir.dt.float16
    Alu = mybir.AluOpType
    Act = mybir.ActivationFunctionType

    nc = bacc.Bacc("TRN2", target_bir_lowering=False, debug=False,
                   num_devices=NCORES)

    # QA: per-tile blocks of [97 QA2 cols | 13 QA1 run cols], f32
    dQA = nc.dram_tensor("QA", (128, HT * (P + NRUNS)), f32,
                         kind="ExternalInput")

    # negated W2 blocks, f16, per hidden tile
    dW2 = nc.dram_tensor("W2TP", (128, HT * P), f16, kind="ExternalInput")
    # US [110, 97] (U2B.T rows 0..96, U1B.T+consts rows 97..109) | M [110, 1261]
    dUM = nc.dram_tensor("UM", (P + NRUNS, P + NLOC), f16,
                         kind="ExternalInput")
    dOUT = nc.dram_tensor("out", (P, NLOC), f16, kind="ExternalOutput")

    b = float(beta)
    c_open = float(np.float32(-b * b / (1.0 - b)))
    b2c = float(np.float32(b * b))
    NU = P + NRUNS  # 110 contraction rows for the linear-part matmul

    with TileContext(nc) as tc:
        with tc.tile_pool(name="const", bufs=1) as cpool, \
             tc.tile_pool(name="work", bufs=2) as wpool, \
             tc.tile_pool(name="psO", bufs=1, space="PSUM") as pso_pool:

            TB = P + NRUNS  # 110-col per-tile block in tQA
            tQA = cpool.tile([128, HT * TB], f32, tag="tQA")
            tW2 = cpool.tile([128, HT * P], f16, tag="tW2")
            tUM = cpool.tile([NU, P + NLOC], f16, tag="tUM")
            qb = cpool.tile([128, HT, NLOC], f32, tag="qb")
            z16 = cpool.tile([128, HT, NLOC], f16, tag="z16")
            st = cpool.tile([128, HT, NLOC], f32, tag="st")

            # ---- loads (sync ring only; scalar queue stays clean);
            # tQA in ONE dma: the column-major chain needs all 4 tiles ----
            nc.sync.dma_start(out=tQA, in_=dQA.ap())
            nc.sync.dma_start(out=tW2, in_=dW2.ap())
            nc.sync.dma_start(out=tUM, in_=dUM.ap())

            # persistent output psum banks
            pso = [pso_pool.tile([P, cw], f32, tag=f"pso{n}", name=f"pso{n}")
                   for n, (c0, cw) in enumerate(CHUNKS)]

            # ---- linear part: pso = US.T @ M (scheduled before any z16) ----
            for n, (c0, cw) in enumerate(CHUNKS):
                nc.tensor.matmul(pso[n], tUM[:, 0:P],
                                 tUM[:, P + c0:P + c0 + cw],
                                 start=True, stop=False)

            # ---- qb build: run 0 on the DVE (so the chain starts ~1us
            # after the first DMA receipt), runs 1..12 on ScalarE
            # (GpSimd tensor_scalar measured at ~2us per op — unusable).
            # Run-major order so the column-major chain below never
            # starves: ScalarE builds ~1.5us/run vs chain ~2.6us/run ----
            def qb_run(t, r, eng):
                dst = qb[:, t, r * P:(r + 1) * P]
                src = tQA[:, t * TB:t * TB + P]
                bias = tQA[:, t * TB + P + r: t * TB + P + r + 1]
                if eng == "dve":
                    nc.vector.tensor_scalar(dst, src, bias, None,
                                            Alu.add, Alu.bypass)
                else:
                    nc.scalar.activation(dst, src, Act.Identity,
                                         bias=bias, scale=1.0)

            for t in range(HT):
                qb_run(t, 0, "dve")
            for r in range(1, NRUNS):
                for t in range(HT):
                    qb_run(t, r, "act")

            ob = cpool.tile([P, NLOC], f16, tag="ob")

            # ---- column-major chain: each group covers a column range of
            # ALL 4 hidden tiles, so psum chunks finish (and store) while
            # the chain is still running.  Graduated group sizes keep the
            # chain just behind the ScalarE qb build.  chunk_done = psum
            # chunk fully covered once this group's z16 is written ----
            cgroups = [(0, 2 * P, None), (2 * P, 3 * P, 0),
                       (5 * P, 4 * P, 1), (9 * P, 4 * P, 2)]
            for (a, w, chunk) in cgroups:
                qs = qb[:, :, a:a + w]
                ss = st[:, :, a:a + w]
                zs = z16[:, :, a:a + w]
                nc.vector._custom_dve(ops["SNN_OPEN"], out=ss, in0=qs,
                                      s0=c_open, s1=b, imm2=b2c)
                for opn in ("SNN_WGZ", "SNN_ZGW", "SNN_WGZ", "SNN_ZGW"):
                    nc.vector._custom_dve(ops[opn], out=ss, in0=ss,
                                          in1=qs, s0=b)
                nc.vector._custom_dve(ops["SNN_WGZ"], out=zs, in0=ss,
                                      in1=qs, s0=b)

                if chunk is None:
                    continue
                c0, cw = CHUNKS[chunk]
                for ti in range(HT):
                    nc.tensor.matmul(
                        pso[chunk], tW2[:, ti * P:(ti + 1) * P],
                        z16[:, ti, c0:c0 + cw],
                        start=False, stop=(ti == HT - 1))
                dst = ob[:, c0:c0 + cw]
                nc.scalar.activation(dst, pso[chunk], Act.Copy)
                nc.sync.dma_start(out=dOUT.ap()[:, c0:c0 + cw], in_=dst)

    if not nc.is_finalized():
        nc.finalize()
    return nc


def _prep_inputs(x, embed_w, W1, b1, W2, b2, beta, thr):
    E = embed_w.astype(np.float64)
    W1d = W1.astype(np.float64)
    A1T = np.ascontiguousarray(E @ W1d[:, :H].T)   # [97, 512] f64
    A2T = np.ascontiguousarray(E @ W1d[:, H:].T)

    kq = -1.0 / ((1.0 - beta) * thr)
    bkq = beta * kq
    # qb = bkq*A2T[j,h] + (bkq*A1T[i,h] + beta*(1 + kq*b1[h]))
    QA2 = (bkq * A2T.T).astype(np.float32)                     # [512, 97]
    QA1 = (bkq * A1T.T
           + (beta * (1.0 + kq * b1.astype(np.float64)))[:, None]
           ).astype(np.float32)                                # [512, 97]

    # linear output part: T += b16*kq*(W2@A1T.T)[:,i] + b16*kq*(W2@A2T.T)[:,j]
    #                        + b16*W2@(1+kq*b1) + G*b2 - b16*W2@1
    W2d = W2.astype(np.float64)
    b16 = beta ** 16
    G = (1.0 - beta ** NSTEPS) / (1.0 - beta)
    U1 = b16 * kq * (W2d @ A1T.T)                              # [97, 97]
    U2 = b16 * kq * (W2d @ A2T.T)                              # [97, 97]
    cvec = (b16 * (W2d @ (1.0 + kq * b1.astype(np.float64)))
            + G * b2.astype(np.float64) - b16 * W2d.sum(axis=1))  # [97]

    # [128, 4*97] f16: h-tile t's NEGATED W2 block in cols [t*97, (t+1)*97)
    W2TP = np.ascontiguousarray(
        (-W2.T).astype(np.float16).reshape(HT, 128, P).transpose(1, 0, 2)
        .reshape(128, HT * P))

    # M [110, 1261]: rows j<97: [c mod 97 == j]; row 97+r: [c//97 == r]
    c = np.arange(NLOC)
    M = np.zeros((P + NRUNS, NLOC), np.float16)
    M[c % P, c] = 1.0
    M[P + c // P, c] = 1.0

    in_maps = []
    TB = P + NRUNS
    for k in range(NCORES):
        ivals = RUN_BASE[k] + np.arange(NRUNS)        # run i-indices, <= 96
        # QA tensor: per-tile blocks [97 QA2 | 13 QA1]
        QAt = np.empty((128, HT * TB), np.float32)
        for t in range(HT):
            hs = slice(t * 128, (t + 1) * 128)
            QAt[:, t * TB:t * TB + P] = QA2[hs]
            QAt[:, t * TB + P:(t + 1) * TB] = QA1[hs][:, ivals]
        # US [110, 97]: rows 0..96 = U2.T; rows 97.. = (U1[:, ivals]+cvec).T
        US = np.empty((P + NRUNS, P), np.float64)
        US[:P] = U2.T
        US[P:] = (U1[:, ivals] + cvec[:, None]).T
        UM = np.concatenate([US.astype(np.float16), M], axis=1)
        in_maps.append({
            "QA": np.ascontiguousarray(QAt),
            "W2TP": W2TP,
            "UM": np.ascontiguousarray(UM),
        })
    return in_maps


def kernel(x, embed_w, W1, b1, W2, b2, beta1, beta2, thr1, thr2, **_):
    from concourse.bass_utils import run_bass_kernel_spmd

    x, embed_w, W1, b1, W2, b2 = (
        np.asarray(a) for a in (x, embed_w, W1, b1, W2, b2))
    beta = float(np.clip(np.float32(beta1), 0.1, 0.9))
    beta2c = float(np.clip(np.float32(beta2), 0.1, 0.9))
    thr = float(max(np.float32(thr1), 0.1))
    assert abs(beta - beta2c) < 1e-12, "kernel assumes beta1 == beta2"

    key = (round(beta, 9), round(thr, 9))
    if key not in _CACHE:
        _CACHE[key] = _build_bass(beta, thr)
    nc = _CACHE[key]

    in_maps = _prep_inputs(x, embed_w, W1, b1, W2, b2, beta, thr)
    res = None
    for attempt in range(3):
        try:
            res = run_bass_kernel_spmd(nc, in_maps,
                                       core_ids=list(range(NCORES)))
            break
        except Exception:
            # rare transient NRT_EXEC_UNIT_UNRECOVERABLE under this
            # runtime; give it a moment and retry
            if attempt == 2:
                raise
            import time
            time.sleep(2.0)

    T = np.empty((P, P * P), np.float32)
    for k in range(NCORES):
        n_runs = (RUN_BASE[k + 1] - RUN_BASE[k]) if k < NCORES - 1 else NRUNS
        g0 = RUN_BASE[k] * P
        T[:, g0:g0 + n_runs * P] = \
            res.results[k]["out"].astype(np.float32)[:, :n_runs * P]

    pid = x[:, 0].astype(np.int64) * P + x[:, 1].astype(np.int64)
    return np.ascontiguousarray(T.T[pid]).astype(np.float32)


# revision 56
# speedup vs baseline: 1.0327x; 1.0327x over previous
"""GrokkingSNN forward on 8 TRN2 NeuronCores — run-aligned, matmul-free
cur1 build, column-major chain with stores pipelined under it.

Math notes
----------
Reference loop (thr clamps absorbed; beta1=beta2=beta):
    m_t = beta*m_{t-1} + c - s_{t-1},  s_t = H(m_t - 1)
    out = W2 @ S + G*b2,  S = sum_t beta^(15-t) s_t
With q = kq*cur1 + (1 + kq*b1), kq = -1/((1-beta)*thr), and the premultiplied
state w = beta*z, qb = beta*q, the spike sum telescopes:
    S = beta^16*q - z16 - beta^16
so the device needs z16 (nonlinear part) while the beta^16*q term is LINEAR
in cur1 and is applied in output space.

x has only 97*97 distinct rows, so the net is evaluated on the (i, j) grid.
cur1[h, c] = A1T[i(c), h] + A2T[j(c), h] with i = c//97, j = c mod 97, hence
qb[:, c] = QA2[:, j] + QA1col(i): a 97-periodic tensor plus a per-run
per-partition column.  Each core takes 13 WHOLE runs of 97 columns (cores
overlap by one run so every core runs the identical run-aligned program), so
qb is built with NO matmuls/one-hots: run 0 via 4 DVE tensor_scalar adds
(so the chain starts ~1us after the first DMA receipt), runs 1-12 as 48
ScalarE activation-with-per-partition-bias ops that stay ~1us/run ahead of
the chain (GpSimd tensor_scalar measured ~2us/op — unusable).

The 15-step recurrence runs as six fused 8-ALU-stage custom-DVE ops
(OPEN, WGZ, ZGW, WGZ, ZGW, WGZ -> z16 fp16).  15 decisions x ~3 ALUs each
over 8 ALU stages/cycle makes ~5 cycles/element the hard DVE floor; the 2x
perf modes duplicate the per-element program across ALU stages, so 8-stage
programs can never engage them — the chain is the ~35us bottleneck and
everything else hides behind it.  Groups are COLUMN-major ([128, 4tiles, w]
strided APs over all hidden tiles) with graduated widths (r0-1 | r2-4 |
r5-8 | r9-12): each group completes whole psum chunks, whose W2 matmuls,
psum->f16 cast and HBM store then run DURING the chain.  Store descriptors
all funnel through one DMA engine (~45-100ns per partition row, regardless
of queue/layout — loads spread across 16 engines by destination partition,
stores do not), so hiding the first two chunk stores under the chain is the
only way to shorten the ~5us/chunk drain tail; only the last chunk pays it.

The linear beta^16*q part costs ONE extra fp16 matmul per psum chunk:
stationary [U2B.T ; U1B.T+consts] against a constant 0/1 matrix M
([j==row] rows and [run==row] rows), scheduled before any z16 exists.
Remaining fixed costs: ~4.7us first-DMA receipt latency and ~8us
runtime teardown (reset of all 256 semaphores + cross-core barriers).
"""

import os
import sys

import numpy as np

for _p in ("/opt/trn_rl_repo",):
    if _p not in sys.path and os.path.isdir(_p):
        sys.path.insert(0, _p)

P = 97          # vocab / output dim
H = 512         # hidden
NSTEPS = 15
NCORES = 8
NRUNS = 13      # 97-col runs per core
NLOC = NRUNS * P            # 1261 grid columns per core
HT = H // 128   # 4 hidden tiles
RUN_BASE = [(k * P) // NCORES for k in range(NCORES)]   # first run per core
CHUNKS = [(0, 5 * P), (5 * P, 4 * P), (9 * P, 4 * P)]  # psum col chunks

_CACHE = {}
_DVE_OPS = {}


def _register_dve_ops():
    """Register the three fused recurrence ops (documented extension
    path: define a DveOp and append to dve_ops.OPS)."""
    if _DVE_OPS:
        return _DVE_OPS
    import concourse.dve_ops as dvo
    from concourse.dve_spec import (Spec, Src0, Src1, C0, C1, C2,
                                    lower, _has_src1)
    from concourse.dve_uop import DveOpSpec

    def f32(a):
        return np.asarray(a, np.float32)

    # OPEN: in0 = qb (= beta*q).  s1 = [qb < c0]; z2 = qb*c1 - c2 - s1;
    # w2 = z2*c1; s2 = [w2 > qb]; z3 = w2 - s2; out = z3*c1   (c1 = beta)
    _s1 = Src0 < C0
    _z2 = (Src0 * C1 - C2) - _s1
    _w2 = _z2 * C1
    _z3 = _w2 - (_w2 > Src0)
    OPEN_spec = Spec(
        body=_z3 * C1,
        reference=lambda in0, in1, s0, s1, imm2: (lambda qb, s1v: (
            lambda z2: (lambda w2: (lambda z3: f32(z3 * s1))(
                f32(w2 - (w2 > qb))))(f32(z2 * s1)))(
            f32(f32(f32(qb * s1) - imm2) - s1v)))(
            f32(in0), f32(in0 < s0)),
    )

    # WGZ: in0 = w_t, in1 = qb: three spike decisions -> z_{t+3}
    _a2 = Src0 - (Src0 > Src1)
    _a3 = _a2 * C0
    _a5 = _a3 - (_a3 > Src1)
    _a6 = _a5 * C0
    WGZ_spec = Spec(
        body=_a6 - (_a6 > Src1),
        reference=lambda in0, in1, s0, s1, imm2: (lambda w, qb, b: (
            lambda z1: (lambda w1: (lambda z2: (lambda w2: f32(w2 - (w2 > qb)))(
                f32(z2 * b)))(f32(w1 - (w1 > qb))))(f32(z1 * b)))(
            f32(w - (w > qb))))(f32(in0), f32(in1), s0),
    )

    # ZGW: in0 = z_t, in1 = qb: two spike decisions -> w_{t+2}
    _b1 = Src0 * C0
    _b3 = _b1 - (_b1 > Src1)
    _b4 = _b3 * C0
    _b6 = _b4 - (_b4 > Src1)
    ZGW_spec = Spec(
        body=_b6 * C0,
        reference=lambda in0, in1, s0, s1, imm2: (lambda z, qb, b: (
            lambda w: (lambda z1: (lambda w1: (lambda z2: f32(z2 * b))(
                f32(w1 - (w1 > qb))))(f32(z1 * b)))(f32(w - (w > qb))))(
            f32(z * b)))(f32(in0), f32(in1), s0),
    )

    for name, spec in (("SNN_OPEN", OPEN_spec), ("SNN_WGZ", WGZ_spec),
                       ("SNN_ZGW", ZGW_spec)):
        if name not in dvo._SUB_OPCODE_FOR_NAME:
            row = dvo._CUSTOM_DVE_ROW_BASE + len(dvo.OPS)
            assert row < 0x20
            dvo._SUB_OPCODE_FOR_NAME[name] = row
            shas = {}
            for ver in ("v3", "v4"):
                uops = lower(spec, ver=ver)
                shas[ver] = DveOpSpec(name=name, opcode=row, uops=uops,
                                      rd1_en=_has_src1(spec)).sha(ver)
            op = dvo.DveOp(name, spec, subdim=False, uops_sha=shas)
            dvo.OPS.append(op)
            dvo.CUSTOM_DVE_SPECS[name] = spec
        _DVE_OPS[name] = next(o for o in dvo.OPS if o.name == name)
    return _DVE_OPS


def _build_bass(beta: float, thr: float):
    from concourse import bacc, mybir
    from concourse.tile import TileContext

    ops = _register_dve_ops()
    f32 = mybir.dt.float32
    f16 = mybir.dt.float16
    Alu = mybir.AluOpType
    Act = mybir.ActivationFunctionType

    nc = bacc.Bacc("TRN2", target_bir_lowering=False, debug=False,
                   num_devices=NCORES)

    # QA: per-tile blocks of [97 QA2 cols | 13 QA1 run cols], f32
    dQA = nc.dram_tensor("QA", (128, HT * (P + NRUNS)), f32,
                         kind="ExternalInput")

    # negated W2 blocks, f16, per hidden tile
    dW2 = nc.dram_tensor("W2TP", (128, HT * P), f16, kind="ExternalInput")
    # US [110, 97] (U2B.T rows 0..96, U1B.T+consts rows 97..109) | M [110, 1261]
    dUM = nc.dram_tensor("UM", (P + NRUNS, P + NLOC), f16,
                         kind="ExternalInput")
    dOUT = nc.dram_tensor("out", (P, NLOC), f16, kind="ExternalOutput")

    b = float(beta)
    c_open = float(np.float32(-b * b / (1.0 - b)))
    b2c = float(np.float32(b * b))
    NU = P + NRUNS  # 110 contraction rows for the linear-part matmul

    with TileContext(nc) as tc:
        with tc.tile_pool(name="const", bufs=1) as cpool, \
             tc.tile_pool(name="work", bufs=2) as wpool, \
             tc.tile_pool(name="psO", bufs=1, space="PSUM") as pso_pool:

            TB = P + NRUNS  # 110-col per-tile block in tQA
            tQA = cpool.tile([128, HT * TB], f32, tag="tQA")
            tW2 = cpool.tile([128, HT * P], f16, tag="tW2")
            tUM = cpool.tile([NU, P + NLOC], f16, tag="tUM")
            qb = cpool.tile([128, HT, NLOC], f32, tag="qb")
            z16 = cpool.tile([128, HT, NLOC], f16, tag="z16")
            st = cpool.tile([128, HT, NLOC], f32, tag="st")

            # ---- loads (sync ring only; scalar queue stays clean);
            # tQA in ONE dma: the column-major chain needs all 4 tiles ----
            nc.sync.dma_start(out=tQA, in_=dQA.ap())
            nc.sync.dma_start(out=tW2, in_=dW2.ap())
            nc.sync.dma_start(out=tUM, in_=dUM.ap())

            # persistent output psum banks
            pso = [pso_pool.tile([P, cw], f32, tag=f"pso{n}", name=f"pso{n}")
                   for n, (c0, cw) in enumerate(CHUNKS)]

            # ---- linear part: pso = US.T @ M (scheduled before any z16) ----
            for n, (c0, cw) in enumerate(CHUNKS):
                nc.tensor.matmul(pso[n], tUM[:, 0:P],
                                 tUM[:, P + c0:P + c0 + cw],
                                 start=True, stop=False)

            # ---- qb build: run 0 on the DVE (so the chain starts ~1us
            # after the first DMA receipt), runs 1..12 on ScalarE
            # (GpSimd tensor_scalar measured at ~2us per op — unusable).
            # Run-major order so the column-major chain below never
            # starves: ScalarE builds ~1.5us/run vs chain ~2.6us/run ----
            def qb_run(t, r, eng):
                dst = qb[:, t, r * P:(r + 1) * P]
                src = tQA[:, t * TB:t * TB + P]
                bias = tQA[:, t * TB + P + r: t * TB + P + r + 1]
                if eng == "dve":
                    nc.vector.tensor_scalar(dst, src, bias, None,
                                            Alu.add, Alu.bypass)
                else:
                    nc.scalar.activation(dst, src, Act.Identity,
                                         bias=bias, scale=1.0)

            for t in range(HT):
                qb_run(t, 0, "dve")
            for r in range(1, NRUNS):
                for t in range(HT):
                    qb_run(t, r, "act")

            ob = cpool.tile([P, NLOC], f16, tag="ob")

            # ---- column-major chain: each group covers a column range of
            # ALL 4 hidden tiles, so psum chunks finish (and store) while
            # the chain is still running.  Graduated group sizes keep the
            # chain just behind the ScalarE qb build.  chunk_done = psum
            # chunk fully covered once this group's z16 is written ----
            # (col_start, width, tile_lo, tile_hi, chunk_done, d_tiles)
            # The final group is split by tiles so chunk 2's first three
            # W2 matmuls run DURING the tile-3 chain instead of after it.
            cgroups = [(0, 2 * P, 0, 4, None, ()),
                       (2 * P, 3 * P, 0, 4, 0, (0, 1, 2, 3)),
                       (5 * P, 4 * P, 0, 4, 1, (0, 1, 2, 3)),
                       (9 * P, 4 * P, 0, 3, None, (0, 1, 2)),
                       (9 * P, 4 * P, 3, 4, 2, (3,))]
            for (a, w, tlo, thi, chunk, dts) in cgroups:
                if thi - tlo == 1:
                    qs = qb[:, tlo, a:a + w]
                    ss = st[:, tlo, a:a + w]
                    zs = z16[:, tlo, a:a + w]
                else:
                    qs = qb[:, tlo:thi, a:a + w]
                    ss = st[:, tlo:thi, a:a + w]
                    zs = z16[:, tlo:thi, a:a + w]
                nc.vector._custom_dve(ops["SNN_OPEN"], out=ss, in0=qs,
                                      s0=c_open, s1=b, imm2=b2c)
                for opn in ("SNN_WGZ", "SNN_ZGW", "SNN_WGZ", "SNN_ZGW"):
                    nc.vector._custom_dve(ops[opn], out=ss, in0=ss,
                                          in1=qs, s0=b)
                nc.vector._custom_dve(ops["SNN_WGZ"], out=zs, in0=ss,
                                      in1=qs, s0=b)

                # stage D for the chunk this group's z16 completes (for the
                # split tail, tiles 0-2 of chunk 2 are emitted by the
                # tiles-0..2 group and overlap the tile-3 chain)
                if dts:
                    c0, cw = CHUNKS[2 if chunk is None else chunk]
                    for ti in dts:
                        nc.tensor.matmul(
                            pso[2 if chunk is None else chunk],
                            tW2[:, ti * P:(ti + 1) * P],
                            z16[:, ti, c0:c0 + cw],
                            start=False, stop=(ti == HT - 1))
                if chunk is None:
                    continue
                c0, cw = CHUNKS[chunk]
                dst = ob[:, c0:c0 + cw]
                nc.scalar.activation(dst, pso[chunk], Act.Copy)
                nc.sync.dma_start(out=dOUT.ap()[:, c0:c0 + cw], in_=dst)

    if not nc.is_finalized():
        nc.finalize()
    return nc


def _prep_inputs(x, embed_w, W1, b1, W2, b2, beta, thr):
    E = embed_w.astype(np.float64)
    W1d = W1.astype(np.float64)
    A1T = np.ascontiguousarray(E @ W1d[:, :H].T)   # [97, 512] f64
    A2T = np.ascontiguousarray(E @ W1d[:, H:].T)

    kq = -1.0 / ((1.0 - beta) * thr)
    bkq = beta * kq
    # qb = bkq*A2T[j,h] + (bkq*A1T[i,h] + beta*(1 + kq*b1[h]))
    QA2 = (bkq * A2T.T).astype(np.float32)                     # [512, 97]
    QA1 = (bkq * A1T.T
           + (beta * (1.0 + kq * b1.astype(np.float64)))[:, None]
           ).astype(np.float32)                                # [512, 97]

    # linear output part: T += b16*kq*(W2@A1T.T)[:,i] + b16*kq*(W2@A2T.T)[:,j]
    #                        + b16*W2@(1+kq*b1) + G*b2 - b16*W2@1
    W2d = W2.astype(np.float64)
    b16 = beta ** 16
    G = (1.0 - beta ** NSTEPS) / (1.0 - beta)
    U1 = b16 * kq * (W2d @ A1T.T)                              # [97, 97]
    U2 = b16 * kq * (W2d @ A2T.T)                              # [97, 97]
    cvec = (b16 * (W2d @ (1.0 + kq * b1.astype(np.float64)))
            + G * b2.astype(np.float64) - b16 * W2d.sum(axis=1))  # [97]

    # [128, 4*97] f16: h-tile t's NEGATED W2 block in cols [t*97, (t+1)*97)
    W2TP = np.ascontiguousarray(
        (-W2.T).astype(np.float16).reshape(HT, 128, P).transpose(1, 0, 2)
        .reshape(128, HT * P))

    # M [110, 1261]: rows j<97: [c mod 97 == j]; row 97+r: [c//97 == r]
    c = np.arange(NLOC)
    M = np.zeros((P + NRUNS, NLOC), np.float16)
    M[c % P, c] = 1.0
    M[P + c // P, c] = 1.0

    in_maps = []
    TB = P + NRUNS
    for k in range(NCORES):
        ivals = RUN_BASE[k] + np.arange(NRUNS)        # run i-indices, <= 96
        # QA tensor: per-tile blocks [97 QA2 | 13 QA1]
        QAt = np.empty((128, HT * TB), np.float32)
        for t in range(HT):
            hs = slice(t * 128, (t + 1) * 128)
            QAt[:, t * TB:t * TB + P] = QA2[hs]
            QAt[:, t * TB + P:(t + 1) * TB] = QA1[hs][:, ivals]
        # US [110, 97]: rows 0..96 = U2.T; rows 97.. = (U1[:, ivals]+cvec).T
        US = np.empty((P + NRUNS, P), np.float64)
        US[:P] = U2.T
        US[P:] = (U1[:, ivals] + cvec[:, None]).T
        UM = np.concatenate([US.astype(np.float16), M], axis=1)
        in_maps.append({
            "QA": np.ascontiguousarray(QAt),
            "W2TP": W2TP,
            "UM": np.ascontiguousarray(UM),
        })
    return in_maps


def kernel(x, embed_w, W1, b1, W2, b2, beta1, beta2, thr1, thr2, **_):
    from concourse.bass_utils import run_bass_kernel_spmd

    x, embed_w, W1, b1, W2, b2 = (
        np.asarray(a) for a in (x, embed_w, W1, b1, W2, b2))
    beta = float(np.clip(np.float32(beta1), 0.1, 0.9))
    beta2c = float(np.clip(np.float32(beta2), 0.1, 0.9))
    thr = float(max(np.float32(thr1), 0.1))
    assert abs(beta - beta2c) < 1e-12, "kernel assumes beta1 == beta2"

    key = (round(beta, 9), round(thr, 9))
    if key not in _CACHE:
        _CACHE[key] = _build_bass(beta, thr)
    nc = _CACHE[key]

    in_maps = _prep_inputs(x, embed_w, W1, b1, W2, b2, beta, thr)
    res = None
    for attempt in range(3):
        try:
            res = run_bass_kernel_spmd(nc, in_maps,
                                       core_ids=list(range(NCORES)))
            break
        except Exception:
            # rare transient NRT_EXEC_UNIT_UNRECOVERABLE under this
            # runtime; give it a moment and retry
            if attempt == 2:
                raise
            import time
            time.sleep(2.0)

    T = np.empty((P, P * P), np.float32)
    for k in range(NCORES):
        n_runs = (RUN_BASE[k + 1] - RUN_BASE[k]) if k < NCORES - 1 else NRUNS
        g0 = RUN_BASE[k] * P
        T[:, g0:g0 + n_runs * P] = \
            res.results[k]["out"].astype(np.float32)[:, :n_runs * P]

    pid = x[:, 0].astype(np.int64) * P + x[:, 1].astype(np.int64)
    return np.ascontiguousarray(T.T[pid]).astype(np.float32)
